# revision 40
# baseline (speedup 1.0000x reference)
"""Trainium2 Bass kernel for weighted-KDE log-density (retrieval_knn).

Math:
  out[b] = logsumexp_n( z_bn ) + hterm_b,   z_bn = 100 x_b.X_n + c_n
  c_n = log_softmax(W)_n - 50 ||X_n||^2,    hterm_b = -50 ||x_b||^2 + log_norm.

At bandwidth 0.1 in 256 dims the logsumexp is dominated by the nearest
coreset point (lse - max <= 0.7 on this data, vs an abs tolerance of ~200),
which permits per-chunk summaries that the host merges exactly in f64.

Device strategy (8 cores, data-parallel over the 8192-query batch):
  * 1024 queries per core as 8 partition-tiles of 128; coreset sorted by c
    and split into 16 chunks of 1024.
  * fp8(e4m3) DoubleRow matmuls: one instruction contracts K=256
    (2 k-tiles x 128 partitions) at 0.5 cycles/row. PSUM accumulates
    y = (32 x).X' + r/3.125 where X' = X - 0.5 and r = c - chat_chunk
    (r rides in as an fp8 hi/lo ones-matmul, also DoubleRow).
  * Per (b-tile, chunk) unit, one of two extraction lanes, strictly
    alternating so the Act and DVE engines run concurrently on the 4-deep
    PSUM ring (both are ~95% busy; they are the bottleneck):
      - Act lane: exp((z' - mhat_b)/K) summed by the activation
        accumulator (exp written back in place to PSUM, which is the
        cheaper access path); mhat is a host-side analytic estimate of
        max_n z (extreme-value model mu + 3.89 sigma), K=16 makes the exp
        window +-1400 so overflow/underflow is impossible.
      - DVE lane: hard max via tensor_reduce.
  * Startup: HWDGE configs and the DMA bus serialize, so the critical
    inputs use the fewest possible DMAs across SP/Act/Pool queues, and
    ~60 tiny warmup matmuls hold the PE p-state at full speed through the
    DMA wait. Results DMA out incrementally to hide the output latency.
  * Host merges per-chunk contributions (max or mhat + K log s) with exact
    f64 logsumexp and adds hterm.
"""

import numpy as np
import ml_dtypes

B, N, D = 8192, 16384, 256
BW = 0.1
NCORES = 8
BLOC = B // NCORES            # 1024 queries per core
P = 128
NBT = BLOC // P               # 8 b-tiles per core
CHN = 1024                    # coreset chunk width
NCH = N // CHN                # 16 chunks
NF = 512                      # matmul free-dim slice (one PSUM bank)

XSCALE = 32.0                 # x quantization scale (|32x| < 240 = e4m3 max)
REST = 100.0 / XSCALE         # remaining scale folded into extraction
K = 16.0                      # softmax temperature for the Act lane
G0 = 3.89                     # EV coefficient for the mhat model

F8 = ml_dtypes.float8_e4m3

_prog_cache = {}


def _lane(j, t):
    """Which engine extracts unit (chunk j, b-tile t): 0 = Act, 1 = DVE."""
    return (j * NBT + t) % 2


# ---------------------------------------------------------------------------
# Workaround: this walrus build rejects instructions carrying more than one
# sync wait ("Too many sync wait commands"). Tile attaches multi-waits to
# instructions. Split them at the BIR-JSON level: move all but the last wait
# of an instruction onto same-engine NoOps inserted just before it.
# ---------------------------------------------------------------------------
_patched = [False]


def _split_multiwaits_json(bir: bytes) -> bytes:
    import json

    d = json.loads(bir)
    uid = [0]
    for fn in d.get("functions", []):
        for blk in fn.get("blocks", []):
            insts = blk.get("instructions", [])
            out = []
            for inst in insts:
                si = inst.get("sync_info")
                waits = si.get("on_wait", []) if si else []
                if len(waits) > 1:
                    for w in waits[:-1]:
                        uid[0] += 1
                        out.append({
                            "debug": inst.get("debug", 0),
                            "engine": inst["engine"],
                            "ins": [],
                            "name": f"{inst['name']}_wsplit{uid[0]}",
                            "opcode": "NoOp",
                            "outs": [],
                            "sync_info": {"on_update": [], "on_wait": [w]},
                        })
                    si["on_wait"] = [waits[-1]]
                out.append(inst)
            blk["instructions"] = out
    return json.dumps(d).encode()


def _apply_patch():
    if _patched[0]:
        return
    from concourse import bass_utils, bass2jax

    orig = bass_utils.compile_bir_kernel

    def wrapped(bir_json, tmpdir, neff_name="file.neff"):
        return orig(_split_multiwaits_json(bir_json), tmpdir, neff_name=neff_name)

    bass_utils.compile_bir_kernel = wrapped
    if getattr(bass2jax, "compile_bir_kernel", None) is orig:
        bass2jax.compile_bir_kernel = wrapped
    _patched[0] = True


# ---------------------------------------------------------------------------


def _build_program():
    import concourse.bass as bass
    import concourse.tile as tile
    from concourse import mybir

    f8 = mybir.dt.float8e4
    f32 = mybir.dt.float32
    Alu = mybir.AluOpType
    Act = mybir.ActivationFunctionType
    DR = mybir.MatmulPerfMode.DoubleRow

    nc = bass.Bass("TRN2", target_bir_lowering=False, debug=False)

    xT = nc.dram_tensor("xT", [P, 2, BLOC], f8, kind="ExternalInput").ap()
    XT = nc.dram_tensor("XT", [P, 2, N], f8, kind="ExternalInput").ap()
    RB = nc.dram_tensor("RB", [1, 2, N], f8, kind="ExternalInput").ap()
    BI = nc.dram_tensor("BI", [P, NBT * NCH], f32, kind="ExternalInput").ap()
    res = nc.dram_tensor("res", [P, NBT * NCH], f32, kind="ExternalOutput").ap()

    with tile.TileContext(nc) as tc:
        with (
            tc.tile_pool(name="xw", bufs=1) as xw_pool,
            tc.tile_pool(name="Xc", bufs=3) as Xc_pool,
            tc.tile_pool(name="rb", bufs=3) as rb_pool,
            tc.tile_pool(name="ps", bufs=4, space="PSUM") as ps_pool,
            tc.tile_pool(name="misc", bufs=1) as misc_pool,
        ):
            # ones rows for the bias matmuls are synthesized on-chip (on the
            # DVE, idle at start); they also feed PE warmup matmuls that keep
            # the tensor engine busy through the initial DMA wait so it is at
            # full p-state for the first real fill
            ones = misc_pool.tile([1, 2, P], f8, tag="ones")
            nc.vector.memset(ones[:], 1.0)

            # startup DMAs: HWDGE configs and the DMA transfer bus are both
            # serialized resources, so use the fewest DMAs on the critical
            # path: Xc0 on SP, xw + bi on the Act HWDGE queue, rb0 on the
            # Pool SWDGE path whose single-partition transfer is spread
            # across the 16 DMA engines (nearly free on the bus)
            Xc0 = Xc_pool.tile([P, 2, CHN], f8, tag="Xc")
            nc.sync.dma_start(Xc0[:], XT[:, :, 0:CHN])
            xw = xw_pool.tile([P, 2, BLOC], f8, tag="xw")
            nc.scalar.dma_start(xw[:], xT[:])
            rb0 = rb_pool.tile([1, 2, CHN], f8, tag="rb")
            nc.gpsimd.dma_start(rb0[:], RB[:, :, 0:CHN])
            bi = misc_pool.tile([P, NBT * NCH], f32, tag="bi")
            nc.scalar.dma_start(bi[:], BI[:])

            resT = misc_pool.tile([P, NBT * NCH], f32, tag="res")

            pwarm = ps_pool.tile([P, CHN], f32, tag="ps")
            for _ in range(59):
                nc.tensor.matmul(
                    pwarm[:, 0:P], ones[:], ones[:],
                    start=True, stop=True, perf_mode=DR,
                )

            for j in range(NCH):
                if j == 0:
                    Xc, rb = Xc0, rb0
                else:
                    rb = rb_pool.tile([1, 2, CHN], f8, tag="rb")
                    nc.gpsimd.dma_start(rb[:], RB[:, :, j * CHN:(j + 1) * CHN])
                    Xc = Xc_pool.tile([P, 2, CHN], f8, tag="Xc")
                    nc.gpsimd.dma_start(Xc[:], XT[:, :, j * CHN:(j + 1) * CHN])

                for t in range(NBT):
                    ps = ps_pool.tile([P, CHN], f32, tag="ps")
                    for nf in range(CHN // NF):
                        sl = slice(nf * NF, (nf + 1) * NF)
                        nc.tensor.matmul(
                            ps[:, sl], xw[:, :, t * P:(t + 1) * P], Xc[:, :, sl],
                            start=True, stop=False, perf_mode=DR,
                        )
                        nc.tensor.matmul(
                            ps[:, sl], ones[:], rb[:, :, sl],
                            start=False, stop=True, perf_mode=DR,
                        )
                    slot = j * NBT + t
                    if _lane(j, t) == 0:
                        # s = sum_n exp((REST*y + chat_j - mhat)/K); exp values
                        # written back in place (PSUM out avoids the slower
                        # SBUF access path on the Act engine)
                        nc.scalar.activation(
                            ps[:], ps[:], Act.Exp,
                            bias=bi[:, slot:slot + 1], scale=REST / K,
                            accum_out=resT[:, slot:slot + 1],
                        )
                    else:
                        # hard max of y over the chunk
                        nc.vector.tensor_reduce(
                            resT[:, slot:slot + 1], ps[:],
                            axis=mybir.AxisListType.X, op=Alu.max,
                        )
                if j == 11:
                    nc.sync.dma_start(res[:, :12 * NBT], resT[:, :12 * NBT])
                elif 11 < j < NCH - 1:
                    lo, hi = j * NBT, (j + 1) * NBT
                    nc.sync.dma_start(res[:, lo:hi], resT[:, lo:hi])
                else:
                    lo, mid, hi = j * NBT, j * NBT + NBT // 2, (j + 1) * NBT
                    nc.sync.dma_start(res[:, lo:mid], resT[:, lo:mid])
                    nc.sync.dma_start(res[:, mid:hi], resT[:, mid:hi])

    return nc


def _host_prep(x, X, W):
    x64 = np.asarray(x, dtype=np.float64)
    X64 = np.asarray(X, dtype=np.float64)
    W64 = np.asarray(W, dtype=np.float64)

    # c_n = log_softmax(W) - 50||X_n||^2, coreset sorted by c
    wmax = W64.max()
    logZ = np.log(np.exp(W64 - wmax).sum()) + wmax
    c = (W64 - logZ) - 50.0 * np.einsum("nd,nd->n", X64, X64)
    order = np.argsort(c)
    Xs = X64[order]
    cs = c[order]

    chat = np.array([cs[j * CHN:(j + 1) * CHN].mean() for j in range(NCH)])
    rres = cs - np.repeat(chat, CHN)

    # mhat: extreme-value estimate of max_n z per query (host-only stats)
    Xbar = X64.mean(0)
    cbar = c.mean()
    Xcent = X64 - Xbar
    ccent = c - cbar
    S_cov = (Xcent.T @ Xcent) / N
    cross = (Xcent.T @ ccent) / N
    var_c = (ccent * ccent).mean()
    mu = 100.0 * x64 @ Xbar + cbar
    var_b = (1e4 * ((x64 @ S_cov) * x64).sum(1)
             + 200.0 * (x64 @ cross) + var_c)
    mhat = mu + G0 * np.sqrt(var_b)

    sx = x64.sum(1)
    log_norm = -(D / 2.0) * np.log(2.0 * np.pi * BW * BW)
    hterm = -50.0 * np.einsum("bd,bd->b", x64, x64) + log_norm

    # fp8 payloads
    def q8(a):
        return a.astype(F8)

    XT_f8 = np.ascontiguousarray(
        q8((Xs - 0.5).T.reshape(2, P, N)).transpose(1, 0, 2)
    )                                                   # [128, 2, N]
    rh = q8(rres / REST)
    rl = q8(rres / REST - rh.astype(np.float64))
    RB_f8 = np.ascontiguousarray(np.stack([rh, rl], 0).reshape(1, 2, N))
    xs8 = q8(XSCALE * x64)                              # [B, D]

    # per-core act bias (chat_j - (mhat - 50 sx)) / K laid out [128, NBT*NCH]
    mdev = mhat - 50.0 * sx                             # mhat in device z'-space
    in_maps = []
    for k in range(NCORES):
        xk = xs8[k * BLOC:(k + 1) * BLOC]               # [BLOC, D]
        xTk = np.ascontiguousarray(
            xk.T.reshape(2, P, BLOC).transpose(1, 0, 2)
        )                                               # [128, 2, BLOC]
        bik = np.empty((P, NBT * NCH), dtype=np.float32)
        md = mdev[k * BLOC:(k + 1) * BLOC].reshape(NBT, P)
        for j in range(NCH):
            for t in range(NBT):
                bik[:, j * NBT + t] = (chat[j] - md[t]) / K
        in_maps.append({
            "xT": xTk, "XT": XT_f8, "RB": RB_f8, "BI": bik,
        })
    aux = {
        "mhat": mhat, "sx": sx, "hterm": hterm, "chat": chat,
    }
    return in_maps, aux


def _host_combine(results, aux):
    mhat = aux["mhat"]
    sx = aux["sx"]
    hterm = aux["hterm"]
    chat = aux["chat"]

    out = np.empty(B, dtype=np.float64)
    for k in range(NCORES):
        r = results[k]["res"].astype(np.float64)        # [P, NBT*NCH]
        base = k * BLOC
        contrib = np.empty((P, NBT, NCH))
        for t in range(NBT):
            bidx = base + t * P + np.arange(P)
            for j in range(NCH):
                v = r[:, j * NBT + t]
                if _lane(j, t) == 0:
                    with np.errstate(divide="ignore"):
                        contrib[:, t, j] = np.where(
                            v > 0.0, mhat[bidx] + K * np.log(v), -np.inf
                        )
                else:
                    contrib[:, t, j] = (REST * v + chat[j]
                                        + 50.0 * sx[bidx])
        m = contrib.max(axis=2)
        tot = np.sum(np.exp(contrib - m[:, :, None]), axis=2)
        lse = m + np.log(tot)                           # [P, NBT]
        out[base:base + BLOC] = lse.T.reshape(BLOC)
    return (out + hterm).astype(np.float32)


def kernel(x, X, W, _trace=False):
    _apply_patch()
    from concourse.bass_utils import run_bass_kernel_spmd

    if "nc" not in _prog_cache:
        _prog_cache["nc"] = _build_program()
    nc = _prog_cache["nc"]

    in_maps, aux = _host_prep(x, X, W)
    br = run_bass_kernel_spmd(
        nc, in_maps, list(range(NCORES)), trace=_trace,
    )
    kernel.last_results = br
    return _host_combine(br.results, aux)


kernel.last_results = None
